# revision 5
# baseline (speedup 1.0000x reference)
"""Gaussian voxel renderer on 8 trn2 NeuronCores.

Math: for voxel p and gaussian n (in input order),
    alpha[p,n] = opa_n * exp(-0.5 * (c_p - mu_n)^T A_n (c_p - mu_n)),  A = inv cov
    w[p,n] = alpha[p,n] * prod_{j<n} (1 - alpha[p,j])
    out[p,:] = sum_n w[p,n] * feat[n,:]

Device pipeline (voxels on partitions, gaussians on the free axis), with the
compositing telescoped to  out = f0 + sum_n S_n * g_n,  S = inclusive
cumprod(1-alpha), g = diff(features):
    u = basis^T @ G            PE, 3-term fp16 split (fp32-grade accuracy)
    alpha = exp(u)             ACT
    m = 1 - alpha              GPSIMD tensor_scalar
    S = cumprod(m)             DVE tensor_tensor_scan, fp32 state, fp16 out
    S^T                        PE fp16 transposes -> PSUM, DVE copy -> SBUF
    r = S^T.T @ g              PE fp16, accumulated over 4 gaussian chunks
Voxel slabs are sharded across the 8 cores; per-gaussian parameters are
replicated. Host does the tiny per-gaussian precompute (quat->rot, 3x3
inverse, fp16 hi/lo splits) in float64 and the final gather/deinterleave.
"""
import numpy as np

import concourse.bacc as bacc
import concourse.tile as tile
import concourse.mybir as mybir
from concourse.bass_utils import run_bass_kernel_spmd
from concourse.masks import make_identity

F32 = mybir.dt.float32
F16 = mybir.dt.float16
AF = mybir.ActivationFunctionType
ALU = mybir.AluOpType

H, W, D = 96, 96, 16
N, F = 512, 32
NCORES = 8
P_TOTAL = H * W * D
P_LOCAL = P_TOTAL // NCORES          # 18432
TILES = P_LOCAL // 128               # 144
NCHUNK = N // 128                    # 4
LO_SCALE = 4096.0                    # 2**12, fp16 low-part scaling

GROUP = 4                            # tiles per r_ps bank / out-copy batch
OUT_CHUNKS = 4                       # output DMA granularity


def _build_nc():
    nc = bacc.Bacc("TRN2", target_bir_lowering=False, debug=False)
    bhi_d = nc.dram_tensor("basis_hi", [10, P_LOCAL], F16, kind="ExternalInput")
    blo_d = nc.dram_tensor("basis_lo", [10, P_LOCAL], F16, kind="ExternalInput")
    ghi_d = nc.dram_tensor("G_hi", [10, N], F16, kind="ExternalInput")
    glo_d = nc.dram_tensor("G_lo", [10, N], F16, kind="ExternalInput")
    ghis_d = nc.dram_tensor("G_his", [10, N], F16, kind="ExternalInput")
    gf_d = nc.dram_tensor("gfeat", [128, NCHUNK * 2 * F], F16, kind="ExternalInput")
    rend_d = nc.dram_tensor("rend", [128, TILES * F], F32, kind="ExternalOutput")

    tpq = TILES // OUT_CHUNKS
    with tile.TileContext(nc) as tc:
        with tc.tile_pool(name="const", bufs=1) as const, \
             tc.tile_pool(name="work", bufs=3) as work, \
             tc.tile_pool(name="outp", bufs=2) as outp, \
             tc.tile_pool(name="ps_u", bufs=2, space="PSUM") as ps_u, \
             tc.tile_pool(name="ps_t", bufs=2, space="PSUM") as ps_t, \
             tc.tile_pool(name="ps_r", bufs=2, space="PSUM") as ps_r:

            bhi_sb = const.tile([10, P_LOCAL], F16)
            nc.sync.dma_start(bhi_sb[:], bhi_d[:])
            blo_sb = const.tile([10, P_LOCAL], F16)
            nc.sync.dma_start(blo_sb[:], blo_d[:])
            ghi_sb = const.tile([10, N], F16)
            nc.sync.dma_start(ghi_sb[:], ghi_d[:])
            glo_sb = const.tile([10, N], F16)
            nc.sync.dma_start(glo_sb[:], glo_d[:])
            ghis_sb = const.tile([10, N], F16)
            nc.sync.dma_start(ghis_sb[:], ghis_d[:])
            gf_sb = const.tile([128, NCHUNK * 2 * F], F16)
            nc.sync.dma_start(gf_sb[:], gf_d[:])
            ident = const.tile([128, 128], F16)
            make_identity(nc, ident[:])

            for q in range(OUT_CHUNKS):
                out_sb = outp.tile([128, tpq * F], F32, tag="out")
                for g in range(tpq // GROUP):
                    r_ps = ps_r.tile([128, GROUP * 2 * F], F32, tag="r")
                    for j in range(GROUP):
                        it = g * GROUP + j
                        i = q * tpq + it
                        sl = slice(i * 128, (i + 1) * 128)
                        u_ps = ps_u.tile([128, N], F32, tag="u")
                        nc.tensor.matmul(u_ps[:], bhi_sb[:, sl], ghi_sb[:],
                                         start=True, stop=False)
                        nc.tensor.matmul(u_ps[:], bhi_sb[:, sl], glo_sb[:],
                                         start=False, stop=False)
                        nc.tensor.matmul(u_ps[:], blo_sb[:, sl], ghis_sb[:],
                                         start=False, stop=True)
                        alpha = work.tile([128, N], F32, tag="alpha")
                        nc.scalar.activation(alpha[:], u_ps[:], AF.Exp)
                        m = work.tile([128, N], F32, tag="m")
                        nc.gpsimd.tensor_scalar(m[:], alpha[:], -1.0, 1.0,
                                                op0=ALU.mult, op1=ALU.add)
                        S = work.tile([128, N], F16, tag="S")
                        nc.vector.tensor_tensor_scan(S[:], m[:], m[:], 1.0,
                                                     op0=ALU.mult,
                                                     op1=ALU.bypass)
                        st_ps = ps_t.tile([128, N], F16, tag="st")
                        for c in range(NCHUNK):
                            nc.tensor.transpose(
                                st_ps[:, c * 128:(c + 1) * 128],
                                S[:, c * 128:(c + 1) * 128], ident[:])
                        ST = work.tile([128, N], F16, tag="ST")
                        nc.scalar.activation(ST[:, 0:256], st_ps[:, 0:256],
                                             AF.Copy)
                        nc.vector.tensor_copy(ST[:, 256:N], st_ps[:, 256:N])
                        for c in range(NCHUNK):
                            nc.tensor.matmul(r_ps[:, j * 2 * F:(j + 1) * 2 * F],
                                             ST[:, c * 128:(c + 1) * 128],
                                             gf_sb[:, c * 2 * F:(c + 1) * 2 * F],
                                             start=(c == 0),
                                             stop=(c == NCHUNK - 1))
                    osl = out_sb[:, g * GROUP * F:(g + 1) * GROUP * F]
                    r3 = r_ps[:].rearrange("p (g two f) -> p (g two f)",
                                           two=2, f=F)
                    hi_view = r_ps[:].rearrange("p (grp two f) -> p grp two f",
                                                two=2, f=F)[:, :, 0, :]
                    lo_view = r_ps[:].rearrange("p (grp two f) -> p grp two f",
                                                two=2, f=F)[:, :, 1, :]
                    nc.scalar.activation(osl, hi_view, AF.Copy)
                    nc.vector.scalar_tensor_tensor(
                        osl, lo_view, 1.0 / LO_SCALE, osl,
                        op0=ALU.mult, op1=ALU.add)
                nc.sync.dma_start(rend_d[:, q * tpq * F:(q + 1) * tpq * F],
                                  out_sb[:])
    nc.compile()
    return nc


_NC_CACHE = None


def _get_nc():
    global _NC_CACHE
    if _NC_CACHE is None:
        _NC_CACHE = _build_nc()
    return _NC_CACHE


def _host_prep(means, scales, rotations, opacities, features, camera_transform,
               coord_grid):
    f8 = np.float64
    means = means.astype(f8)
    scales = scales.astype(f8)
    q = rotations.astype(f8)
    opa = opacities.astype(f8)[:, 0]
    T = camera_transform.astype(f8)

    homo = np.concatenate([means, np.ones((N, 1))], axis=1) @ T.T
    mu = homo[:, :3] / homo[:, 3:4]

    q = q / np.linalg.norm(q, axis=1, keepdims=True)
    w, x, y, z = q[:, 0], q[:, 1], q[:, 2], q[:, 3]
    R = np.stack([
        np.stack([1 - 2 * (y * y + z * z), 2 * (x * y - w * z), 2 * (x * z + w * y)], 1),
        np.stack([2 * (x * y + w * z), 1 - 2 * (x * x + z * z), 2 * (y * z - w * x)], 1),
        np.stack([2 * (x * z - w * y), 2 * (y * z + w * x), 1 - 2 * (x * x + y * y)], 1),
    ], axis=1)
    RS = R * scales[:, None, :]
    cov = np.einsum('nik,njk->nij', RS, RS)
    A = np.linalg.inv(cov)

    Am = np.einsum('nij,nj->ni', A, mu)
    const = -0.5 * np.einsum('ni,ni->n', mu, Am) + np.log(np.maximum(opa, 1e-300))
    G = np.empty((10, N), f8)
    G[0] = -0.5 * A[:, 0, 0]
    G[1] = -0.5 * A[:, 1, 1]
    G[2] = -0.5 * A[:, 2, 2]
    G[3] = -A[:, 0, 1]
    G[4] = -A[:, 0, 2]
    G[5] = -A[:, 1, 2]
    G[6] = Am[:, 0]
    G[7] = Am[:, 1]
    G[8] = Am[:, 2]
    G[9] = np.maximum(const, -60000.0)   # keep within fp16 range

    coords = coord_grid.astype(f8).reshape(-1, 3)
    cx, cy, cz = coords[:, 0], coords[:, 1], coords[:, 2]
    basis = np.stack([cx * cx, cy * cy, cz * cz, cx * cy, cx * cz, cy * cz,
                      cx, cy, cz, np.ones_like(cx)], axis=0)  # [10, P]

    h16 = np.float16
    b_hi = basis.astype(h16)
    b_lo = ((basis - b_hi.astype(f8)) * LO_SCALE).astype(h16)
    G_hi = G.astype(h16)
    G_lo = (G - G_hi.astype(f8)).astype(h16)
    G_his = (G_hi.astype(f8) / LO_SCALE).astype(h16)

    feats = features.astype(f8)
    g = np.empty_like(feats)
    g[:-1] = feats[1:] - feats[:-1]
    g[-1] = -feats[-1]
    g_dev = g.reshape(NCHUNK, 128, F).transpose(1, 0, 2)      # [128, NCHUNK, F]
    gf_hi = g_dev.astype(h16)
    gf_lo = ((g_dev - gf_hi.astype(f8)) * LO_SCALE).astype(h16)
    gf = np.ascontiguousarray(
        np.concatenate([gf_hi[:, :, None, :], gf_lo[:, :, None, :]], axis=2)
        .reshape(128, NCHUNK * 2 * F))
    f0 = feats[0]

    return b_hi, b_lo, G_hi, G_lo, G_his, gf, f0.astype(np.float32)


def kernel(means, scales, rotations, opacities, features, camera_transform,
           coord_grid):
    b_hi, b_lo, G_hi, G_lo, G_his, gf, f0 = _host_prep(
        means, scales, rotations, opacities, features, camera_transform,
        coord_grid)
    nc = _get_nc()
    in_maps = []
    for c in range(NCORES):
        sl = slice(c * P_LOCAL, (c + 1) * P_LOCAL)
        in_maps.append({
            "basis_hi": np.ascontiguousarray(b_hi[:, sl]),
            "basis_lo": np.ascontiguousarray(b_lo[:, sl]),
            "G_hi": G_hi, "G_lo": G_lo, "G_his": G_his, "gfeat": gf,
        })
    res = run_bass_kernel_spmd(nc, in_maps, core_ids=list(range(NCORES)))
    parts = []
    for c in range(NCORES):
        r = res.results[c]["rend"]                      # [128, TILES*F]
        part = r.reshape(128, TILES, F).transpose(1, 0, 2).reshape(P_LOCAL, F)
        parts.append(part)
    out = np.concatenate(parts, axis=0) + f0[None, :]
    return out.reshape(H, W, D, F).astype(np.float32)


# revision 9
# speedup vs baseline: 1.0647x; 1.0647x over previous
"""Gaussian voxel renderer on 8 trn2 NeuronCores.

Math: for voxel p and gaussian n (in input order),
    alpha[p,n] = opa_n * exp(-0.5 * (c_p - mu_n)^T A_n (c_p - mu_n)),  A = inv cov
    w[p,n] = alpha[p,n] * prod_{j<n} (1 - alpha[p,j])
    out[p,:] = sum_n w[p,n] * feat[n,:]

Device pipeline (voxels on partitions, gaussians on the free axis), with the
compositing telescoped to  out = f0 + sum_n S_n * g_n,  S = inclusive
cumprod(1-alpha), g = diff(features):
    u = basis^T @ G            PE, 3-term fp16 split (fp32-grade accuracy)
    alpha = exp(u)             ACT
    m = 1 - alpha              GPSIMD/DVE tensor_scalar (split by tile)
    S = cumprod(m)             DVE/GPSIMD tensor_tensor_scan, fp32 state, fp16 out
    S^T                        PE fp16 transposes -> PSUM, ACT/DVE copy -> SBUF
    r = S^T.T @ [g_hi|g_lo]    PE fp16, accumulated over 4 gaussian chunks
Tiles are processed in pairs to amortize instruction overheads. Voxel slabs
are sharded across the 8 cores; per-gaussian parameters are replicated. Host
does the tiny per-gaussian precompute (quat->rot, 3x3 inverse, fp16 hi/lo
splits) in float64 and the final gather/deinterleave.
"""
import numpy as np

import concourse.bacc as bacc
import concourse.tile as tile
import concourse.mybir as mybir
from concourse.bass_utils import run_bass_kernel_spmd
from concourse.masks import make_identity

F32 = mybir.dt.float32
F16 = mybir.dt.float16
AF = mybir.ActivationFunctionType
ALU = mybir.AluOpType

H, W, D = 96, 96, 16
N, F = 512, 32
NCORES = 8
P_TOTAL = H * W * D
P_LOCAL = P_TOTAL // NCORES          # 18432
TILES = P_LOCAL // 128               # 144
NCHUNK = N // 128                    # 4
LO_SCALE = 4096.0                    # 2**12, fp16 low-part scaling

# tunables (balanced via TimelineSim sweep)
GROUP = 6          # tiles per r_ps bank / out-copy batch (divides tpq=36)
OUT_CHUNKS = 4     # output DMA granularity
ACT_ST = 0       # columns (of 2N per tile-pair) of the S^T copy done by ACT
POOL_SCAN = 0      # of 8 consecutive tiles, how many run the scan on GPSIMD
POOL_OM = 8        # of 8 consecutive tiles, how many run 1-alpha on GPSIMD


def _build_nc(act_st=None, pool_scan=None, pool_om=None, group=None, wbufs=3, ubufs=2):
    act_st = ACT_ST if act_st is None else act_st
    pool_scan = POOL_SCAN if pool_scan is None else pool_scan
    pool_om = POOL_OM if pool_om is None else pool_om
    group = GROUP if group is None else group

    nc = bacc.Bacc("TRN2", target_bir_lowering=False, debug=False)
    bcat_d = nc.dram_tensor("basis_cat", [30, P_LOCAL], F16,
                            kind="ExternalInput")
    gcat_d = nc.dram_tensor("G_cat", [30, N], F16, kind="ExternalInput")
    gf_d = nc.dram_tensor("gfeat", [128, NCHUNK * 2 * F], F16,
                          kind="ExternalInput")
    rend_d = nc.dram_tensor("rend", [128, TILES * F], F32, kind="ExternalOutput")

    tpq = TILES // OUT_CHUNKS
    with tile.TileContext(nc) as tc:
        with tc.tile_pool(name="const", bufs=1) as const, \
             tc.tile_pool(name="work", bufs=wbufs) as work, \
             tc.tile_pool(name="outp", bufs=2) as outp, \
             tc.tile_pool(name="ps_u", bufs=ubufs, space="PSUM") as ps_u, \
             tc.tile_pool(name="ps_t", bufs=2, space="PSUM") as ps_t, \
             tc.tile_pool(name="ps_r", bufs=2, space="PSUM") as ps_r:

            bcat_sb = const.tile([30, P_LOCAL], F16)
            nc.sync.dma_start(bcat_sb[:], bcat_d[:])
            gcat_sb = const.tile([30, N], F16)
            nc.sync.dma_start(gcat_sb[:], gcat_d[:])
            gf_sb = const.tile([128, NCHUNK * 2 * F], F16)
            nc.sync.dma_start(gf_sb[:], gf_d[:])
            ident = const.tile([128, 128], F16)
            make_identity(nc, ident[:])

            for q in range(OUT_CHUNKS):
                out_sb = outp.tile([128, tpq * F], F32, tag="out")
                for grp in range(tpq // group):
                    r_ps = ps_r.tile([128, group * 2 * F], F32, tag="r")
                    for pj in range(group // 2):
                        # process a pair of tiles together
                        jj = [grp * group + 2 * pj, grp * group + 2 * pj + 1]
                        ii = [q * tpq + j for j in jj]
                        u_ps = ps_u.tile([128, 2 * N], F32, tag="u")
                        for k in (0, 1):
                            sl = slice(ii[k] * 128, (ii[k] + 1) * 128)
                            nc.tensor.matmul(u_ps[:, k * N:(k + 1) * N],
                                             bcat_sb[:, sl], gcat_sb[:],
                                             start=True, stop=True)
                        alpha = work.tile([128, 2 * N], F32, tag="alpha")
                        nc.scalar.activation(alpha[:], u_ps[:], AF.Exp)
                        m = work.tile([128, 2 * N], F32, tag="m")
                        if (ii[0] // 2) % 4 < pool_om // 2:
                            nc.gpsimd.tensor_scalar(m[:], alpha[:], -1.0, 1.0,
                                                    op0=ALU.mult, op1=ALU.add)
                        else:
                            nc.vector.tensor_scalar(m[:], alpha[:], -1.0, 1.0,
                                                    op0=ALU.mult, op1=ALU.add)
                        S = work.tile([128, 2 * N], F16, tag="S")
                        for k in (0, 1):
                            on_pool = (ii[k] % 8) >= 8 - pool_scan
                            eng = nc.gpsimd if on_pool else nc.vector
                            eng.tensor_tensor_scan(
                                S[:, k * N:(k + 1) * N],
                                m[:, k * N:(k + 1) * N],
                                m[:, k * N:(k + 1) * N], 1.0,
                                op0=ALU.mult, op1=ALU.bypass)
                        st_ps = ps_t.tile([128, 2 * N], F16, tag="st")
                        for c in range(2 * NCHUNK):
                            nc.tensor.transpose(
                                st_ps[:, c * 128:(c + 1) * 128],
                                S[:, c * 128:(c + 1) * 128], ident[:])
                        ST = work.tile([128, 2 * N], F16, tag="ST")
                        if act_st > 0:
                            nc.scalar.activation(ST[:, 0:act_st],
                                                 st_ps[:, 0:act_st], AF.Copy)
                        if act_st < 2 * N:
                            nc.vector.tensor_copy(ST[:, act_st:2 * N],
                                                  st_ps[:, act_st:2 * N])
                        for k in (0, 1):
                            j = jj[k]
                            for c in range(NCHUNK):
                                nc.tensor.matmul(
                                    r_ps[:, (j % group) * 2 * F:
                                         (j % group + 1) * 2 * F],
                                    ST[:, (k * NCHUNK + c) * 128:
                                       (k * NCHUNK + c + 1) * 128],
                                    gf_sb[:, c * 2 * F:(c + 1) * 2 * F],
                                    start=(c == 0), stop=(c == NCHUNK - 1))
                    osl = out_sb[:, grp * group * F:(grp + 1) * group * F]
                    hi_view = r_ps[:].rearrange("p (grp two f) -> p grp two f",
                                                two=2, f=F)[:, :, 0, :]
                    lo_view = r_ps[:].rearrange("p (grp two f) -> p grp two f",
                                                two=2, f=F)[:, :, 1, :]
                    nc.scalar.activation(osl, hi_view, AF.Copy)
                    nc.vector.scalar_tensor_tensor(
                        osl, lo_view, 1.0 / LO_SCALE, osl,
                        op0=ALU.mult, op1=ALU.add)
                nc.sync.dma_start(rend_d[:, q * tpq * F:(q + 1) * tpq * F],
                                  out_sb[:])
    nc.compile()
    return nc


_NC_CACHE = None


def _get_nc():
    global _NC_CACHE
    if _NC_CACHE is None:
        _NC_CACHE = _build_nc()
    return _NC_CACHE


def _host_prep(means, scales, rotations, opacities, features, camera_transform,
               coord_grid):
    f8 = np.float64
    means = means.astype(f8)
    scales = scales.astype(f8)
    q = rotations.astype(f8)
    opa = opacities.astype(f8)[:, 0]
    T = camera_transform.astype(f8)

    homo = np.concatenate([means, np.ones((N, 1))], axis=1) @ T.T
    mu = homo[:, :3] / homo[:, 3:4]

    q = q / np.linalg.norm(q, axis=1, keepdims=True)
    w, x, y, z = q[:, 0], q[:, 1], q[:, 2], q[:, 3]
    R = np.stack([
        np.stack([1 - 2 * (y * y + z * z), 2 * (x * y - w * z), 2 * (x * z + w * y)], 1),
        np.stack([2 * (x * y + w * z), 1 - 2 * (x * x + z * z), 2 * (y * z - w * x)], 1),
        np.stack([2 * (x * z - w * y), 2 * (y * z + w * x), 1 - 2 * (x * x + y * y)], 1),
    ], axis=1)
    RS = R * scales[:, None, :]
    cov = np.einsum('nik,njk->nij', RS, RS)
    A = np.linalg.inv(cov)

    Am = np.einsum('nij,nj->ni', A, mu)
    const = -0.5 * np.einsum('ni,ni->n', mu, Am) + np.log(np.maximum(opa, 1e-300))
    G = np.empty((10, N), f8)
    G[0] = -0.5 * A[:, 0, 0]
    G[1] = -0.5 * A[:, 1, 1]
    G[2] = -0.5 * A[:, 2, 2]
    G[3] = -A[:, 0, 1]
    G[4] = -A[:, 0, 2]
    G[5] = -A[:, 1, 2]
    G[6] = Am[:, 0]
    G[7] = Am[:, 1]
    G[8] = Am[:, 2]
    G[9] = np.maximum(const, -60000.0)   # keep within fp16 range

    coords = coord_grid.astype(f8).reshape(-1, 3)
    cx, cy, cz = coords[:, 0], coords[:, 1], coords[:, 2]
    basis = np.stack([cx * cx, cy * cy, cz * cz, cx * cy, cx * cz, cy * cz,
                      cx, cy, cz, np.ones_like(cx)], axis=0)  # [10, P]

    h16 = np.float16
    b_hi = basis.astype(h16)
    b_lo = ((basis - b_hi.astype(f8)) * LO_SCALE).astype(h16)
    G_hi = G.astype(h16)
    G_lo = (G - G_hi.astype(f8)).astype(h16)
    G_his = (G_hi.astype(f8) / LO_SCALE).astype(h16)
    b_cat = np.concatenate([b_hi, b_hi, b_lo], axis=0)       # [30, P]
    G_cat = np.concatenate([G_hi, G_lo, G_his], axis=0)      # [30, N]

    feats = features.astype(f8)
    g = np.empty_like(feats)
    g[:-1] = feats[1:] - feats[:-1]
    g[-1] = -feats[-1]
    g_dev = g.reshape(NCHUNK, 128, F).transpose(1, 0, 2)      # [128, NCHUNK, F]
    gf_hi = g_dev.astype(h16)
    gf_lo = ((g_dev - gf_hi.astype(f8)) * LO_SCALE).astype(h16)
    gf = np.ascontiguousarray(
        np.concatenate([gf_hi[:, :, None, :], gf_lo[:, :, None, :]], axis=2)
        .reshape(128, NCHUNK * 2 * F))
    f0 = feats[0]

    return b_cat, G_cat, gf, f0.astype(np.float32)


def kernel(means, scales, rotations, opacities, features, camera_transform,
           coord_grid):
    b_cat, G_cat, gf, f0 = _host_prep(
        means, scales, rotations, opacities, features, camera_transform,
        coord_grid)
    nc = _get_nc()
    in_maps = []
    for c in range(NCORES):
        sl = slice(c * P_LOCAL, (c + 1) * P_LOCAL)
        in_maps.append({
            "basis_cat": np.ascontiguousarray(b_cat[:, sl]),
            "G_cat": G_cat, "gfeat": gf,
        })
    res = run_bass_kernel_spmd(nc, in_maps, core_ids=list(range(NCORES)))
    parts = []
    for c in range(NCORES):
        r = res.results[c]["rend"]                      # [128, TILES*F]
        part = r.reshape(128, TILES, F).transpose(1, 0, 2).reshape(P_LOCAL, F)
        parts.append(part)
    out = np.concatenate(parts, axis=0) + f0[None, :]
    return out.reshape(H, W, D, F).astype(np.float32)


# revision 11
# speedup vs baseline: 1.2058x; 1.1325x over previous
"""Gaussian voxel renderer on 8 trn2 NeuronCores.

Math: for voxel p and gaussian n (in input order),
    alpha[p,n] = opa_n * exp(-0.5 * (c_p - mu_n)^T A_n (c_p - mu_n)),  A = inv cov
    w[p,n] = alpha[p,n] * prod_{j<n} (1 - alpha[p,j])
    out[p,:] = sum_n w[p,n] * feat[n,:]

Device pipeline (voxels on partitions, gaussians on the free axis), with the
compositing telescoped to  out = f0 + sum_n S_n * g_n,  S = inclusive
cumprod(1-alpha), g = diff(features):
    u = basis^T @ G            PE, 3-term fp16 split (fp32-grade accuracy)
    alpha = exp(u)             ACT
    m = 1 - alpha              GPSIMD/DVE tensor_scalar (split by tile)
    S = cumprod(m)             DVE/GPSIMD tensor_tensor_scan, fp32 state, fp16 out
    S^T                        PE fp16 transposes -> PSUM, ACT/DVE copy -> SBUF
    r = S^T.T @ [g_hi|g_lo]    PE fp16, accumulated over 4 gaussian chunks
Tiles are processed in pairs to amortize instruction overheads. Voxel slabs
are sharded across the 8 cores; per-gaussian parameters are replicated. Host
does the tiny per-gaussian precompute (quat->rot, 3x3 inverse, fp16 hi/lo
splits) in float64 and the final gather/deinterleave.
"""
import numpy as np

import concourse.bacc as bacc
import concourse.tile as tile
import concourse.mybir as mybir
from concourse.bass_utils import run_bass_kernel_spmd
from concourse.masks import make_identity

F32 = mybir.dt.float32
F16 = mybir.dt.float16
AF = mybir.ActivationFunctionType
ALU = mybir.AluOpType

H, W, D = 96, 96, 16
N, F = 512, 32
NCORES = 8
P_TOTAL = H * W * D
P_LOCAL = P_TOTAL // NCORES          # 18432
TILES = P_LOCAL // 128               # 144
NCHUNK = N // 128                    # 4
LO_SCALE = 4096.0                    # 2**12, fp16 low-part scaling

# tunables (balanced via TimelineSim sweep)
GROUP = 6          # tiles per r_ps bank / out-copy batch (divides tpq=36)
OUT_CHUNKS = 4     # output DMA granularity
ACT_ST = 384      # columns (of 2N per tile-pair) of the S^T copy done by ACT
POOL_SCAN = 0      # GPSIMD scan rejected by compiler - keep on DVE
POOL_OM = 8        # of 8 consecutive tiles, how many run 1-alpha on GPSIMD


def _build_nc(act_st=None, pool_scan=None, pool_om=None, group=None, wbufs=3, ubufs=2):
    act_st = ACT_ST if act_st is None else act_st
    pool_scan = POOL_SCAN if pool_scan is None else pool_scan
    pool_om = POOL_OM if pool_om is None else pool_om
    group = GROUP if group is None else group

    nc = bacc.Bacc("TRN2", target_bir_lowering=False, debug=False)
    bcat_d = nc.dram_tensor("basis_cat", [30, P_LOCAL], F16,
                            kind="ExternalInput")
    gcat_d = nc.dram_tensor("G_cat", [30, N], F16, kind="ExternalInput")
    gf_d = nc.dram_tensor("gfeat", [128, NCHUNK * 2 * F], F16,
                          kind="ExternalInput")
    rend_d = nc.dram_tensor("rend", [128, TILES * F], F32, kind="ExternalOutput")

    tpq = TILES // OUT_CHUNKS
    with tile.TileContext(nc) as tc:
        with tc.tile_pool(name="const", bufs=1) as const, \
             tc.tile_pool(name="work", bufs=wbufs) as work, \
             tc.tile_pool(name="outp", bufs=2) as outp, \
             tc.tile_pool(name="ps_u", bufs=ubufs, space="PSUM") as ps_u, \
             tc.tile_pool(name="ps_t", bufs=2, space="PSUM") as ps_t, \
             tc.tile_pool(name="ps_r", bufs=2, space="PSUM") as ps_r:

            bcat_sb = const.tile([30, P_LOCAL], F16)
            nc.sync.dma_start(bcat_sb[:], bcat_d[:])
            gcat_sb = const.tile([30, N], F16)
            nc.sync.dma_start(gcat_sb[:], gcat_d[:])
            gf_sb = const.tile([128, NCHUNK * 2 * F], F16)
            nc.sync.dma_start(gf_sb[:], gf_d[:])
            ident = const.tile([128, 128], F16)
            make_identity(nc, ident[:])

            for q in range(OUT_CHUNKS):
                out_sb = outp.tile([128, tpq * F], F32, tag="out")
                for grp in range(tpq // group):
                    r_ps = ps_r.tile([128, group * 2 * F], F32, tag="r")
                    for pj in range(group // 2):
                        # process a pair of tiles together
                        jj = [grp * group + 2 * pj, grp * group + 2 * pj + 1]
                        ii = [q * tpq + j for j in jj]
                        u_ps = ps_u.tile([128, 2 * N], F32, tag="u")
                        for k in (0, 1):
                            sl = slice(ii[k] * 128, (ii[k] + 1) * 128)
                            nc.tensor.matmul(u_ps[:, k * N:(k + 1) * N],
                                             bcat_sb[:, sl], gcat_sb[:],
                                             start=True, stop=True)
                        alpha = work.tile([128, 2 * N], F32, tag="alpha")
                        nc.scalar.activation(alpha[:], u_ps[:], AF.Exp)
                        m = work.tile([128, 2 * N], F32, tag="m")
                        if (ii[0] // 2) % 4 < pool_om // 2:
                            nc.gpsimd.tensor_scalar(m[:], alpha[:], -1.0, 1.0,
                                                    op0=ALU.mult, op1=ALU.add)
                        else:
                            nc.vector.tensor_scalar(m[:], alpha[:], -1.0, 1.0,
                                                    op0=ALU.mult, op1=ALU.add)
                        S = work.tile([128, 2 * N], F16, tag="S")
                        for k in (0, 1):
                            on_pool = (ii[k] % 8) >= 8 - pool_scan
                            eng = nc.gpsimd if on_pool else nc.vector
                            eng.tensor_tensor_scan(
                                S[:, k * N:(k + 1) * N],
                                m[:, k * N:(k + 1) * N],
                                m[:, k * N:(k + 1) * N], 1.0,
                                op0=ALU.mult, op1=ALU.bypass)
                        st_ps = ps_t.tile([128, 2 * N], F16, tag="st")
                        for c in range(2 * NCHUNK):
                            nc.tensor.transpose(
                                st_ps[:, c * 128:(c + 1) * 128],
                                S[:, c * 128:(c + 1) * 128], ident[:])
                        ST = work.tile([128, 2 * N], F16, tag="ST")
                        if act_st > 0:
                            nc.scalar.activation(ST[:, 0:act_st],
                                                 st_ps[:, 0:act_st], AF.Copy)
                        if act_st < 2 * N:
                            nc.vector.tensor_copy(ST[:, act_st:2 * N],
                                                  st_ps[:, act_st:2 * N])
                        for k in (0, 1):
                            j = jj[k]
                            for c in range(NCHUNK):
                                nc.tensor.matmul(
                                    r_ps[:, (j % group) * 2 * F:
                                         (j % group + 1) * 2 * F],
                                    ST[:, (k * NCHUNK + c) * 128:
                                       (k * NCHUNK + c + 1) * 128],
                                    gf_sb[:, c * 2 * F:(c + 1) * 2 * F],
                                    start=(c == 0), stop=(c == NCHUNK - 1))
                    osl = out_sb[:, grp * group * F:(grp + 1) * group * F]
                    hi_view = r_ps[:].rearrange("p (grp two f) -> p grp two f",
                                                two=2, f=F)[:, :, 0, :]
                    lo_view = r_ps[:].rearrange("p (grp two f) -> p grp two f",
                                                two=2, f=F)[:, :, 1, :]
                    nc.scalar.activation(osl, hi_view, AF.Copy)
                    nc.vector.scalar_tensor_tensor(
                        osl, lo_view, 1.0 / LO_SCALE, osl,
                        op0=ALU.mult, op1=ALU.add)
                nc.sync.dma_start(rend_d[:, q * tpq * F:(q + 1) * tpq * F],
                                  out_sb[:])
    nc.compile()
    return nc


_NC_CACHE = None


def _get_nc():
    global _NC_CACHE
    if _NC_CACHE is None:
        _NC_CACHE = _build_nc()
    return _NC_CACHE


def _host_prep(means, scales, rotations, opacities, features, camera_transform,
               coord_grid):
    f8 = np.float64
    means = means.astype(f8)
    scales = scales.astype(f8)
    q = rotations.astype(f8)
    opa = opacities.astype(f8)[:, 0]
    T = camera_transform.astype(f8)

    homo = np.concatenate([means, np.ones((N, 1))], axis=1) @ T.T
    mu = homo[:, :3] / homo[:, 3:4]

    q = q / np.linalg.norm(q, axis=1, keepdims=True)
    w, x, y, z = q[:, 0], q[:, 1], q[:, 2], q[:, 3]
    R = np.stack([
        np.stack([1 - 2 * (y * y + z * z), 2 * (x * y - w * z), 2 * (x * z + w * y)], 1),
        np.stack([2 * (x * y + w * z), 1 - 2 * (x * x + z * z), 2 * (y * z - w * x)], 1),
        np.stack([2 * (x * z - w * y), 2 * (y * z + w * x), 1 - 2 * (x * x + y * y)], 1),
    ], axis=1)
    RS = R * scales[:, None, :]
    cov = np.einsum('nik,njk->nij', RS, RS)
    A = np.linalg.inv(cov)

    Am = np.einsum('nij,nj->ni', A, mu)
    const = -0.5 * np.einsum('ni,ni->n', mu, Am) + np.log(np.maximum(opa, 1e-300))
    G = np.empty((10, N), f8)
    G[0] = -0.5 * A[:, 0, 0]
    G[1] = -0.5 * A[:, 1, 1]
    G[2] = -0.5 * A[:, 2, 2]
    G[3] = -A[:, 0, 1]
    G[4] = -A[:, 0, 2]
    G[5] = -A[:, 1, 2]
    G[6] = Am[:, 0]
    G[7] = Am[:, 1]
    G[8] = Am[:, 2]
    G[9] = np.maximum(const, -60000.0)   # keep within fp16 range

    coords = coord_grid.astype(f8).reshape(-1, 3)
    cx, cy, cz = coords[:, 0], coords[:, 1], coords[:, 2]
    basis = np.stack([cx * cx, cy * cy, cz * cz, cx * cy, cx * cz, cy * cz,
                      cx, cy, cz, np.ones_like(cx)], axis=0)  # [10, P]

    h16 = np.float16
    b_hi = basis.astype(h16)
    b_lo = ((basis - b_hi.astype(f8)) * LO_SCALE).astype(h16)
    G_hi = G.astype(h16)
    G_lo = (G - G_hi.astype(f8)).astype(h16)
    G_his = (G_hi.astype(f8) / LO_SCALE).astype(h16)
    b_cat = np.concatenate([b_hi, b_hi, b_lo], axis=0)       # [30, P]
    G_cat = np.concatenate([G_hi, G_lo, G_his], axis=0)      # [30, N]

    feats = features.astype(f8)
    g = np.empty_like(feats)
    g[:-1] = feats[1:] - feats[:-1]
    g[-1] = -feats[-1]
    g_dev = g.reshape(NCHUNK, 128, F).transpose(1, 0, 2)      # [128, NCHUNK, F]
    gf_hi = g_dev.astype(h16)
    gf_lo = ((g_dev - gf_hi.astype(f8)) * LO_SCALE).astype(h16)
    gf = np.ascontiguousarray(
        np.concatenate([gf_hi[:, :, None, :], gf_lo[:, :, None, :]], axis=2)
        .reshape(128, NCHUNK * 2 * F))
    f0 = feats[0]

    return b_cat, G_cat, gf, f0.astype(np.float32)


def kernel(means, scales, rotations, opacities, features, camera_transform,
           coord_grid):
    b_cat, G_cat, gf, f0 = _host_prep(
        means, scales, rotations, opacities, features, camera_transform,
        coord_grid)
    nc = _get_nc()
    in_maps = []
    for c in range(NCORES):
        sl = slice(c * P_LOCAL, (c + 1) * P_LOCAL)
        in_maps.append({
            "basis_cat": np.ascontiguousarray(b_cat[:, sl]),
            "G_cat": G_cat, "gfeat": gf,
        })
    res = run_bass_kernel_spmd(nc, in_maps, core_ids=list(range(NCORES)))
    parts = []
    for c in range(NCORES):
        r = res.results[c]["rend"]                      # [128, TILES*F]
        part = r.reshape(128, TILES, F).transpose(1, 0, 2).reshape(P_LOCAL, F)
        parts.append(part)
    out = np.concatenate(parts, axis=0) + f0[None, :]
    return out.reshape(H, W, D, F).astype(np.float32)


# revision 12
# speedup vs baseline: 1.3058x; 1.0829x over previous
"""Gaussian voxel renderer on 8 trn2 NeuronCores.

Math: for voxel p and gaussian n (in input order),
    alpha[p,n] = opa_n * exp(-0.5 * (c_p - mu_n)^T A_n (c_p - mu_n)),  A = inv cov
    w[p,n] = alpha[p,n] * prod_{j<n} (1 - alpha[p,j])
    out[p,:] = sum_n w[p,n] * feat[n,:]

Device pipeline (voxels on partitions, gaussians on the free axis), with the
compositing telescoped to  out = f0 + sum_n S_n * g_n,  S = inclusive
cumprod(1-alpha), g = diff(features):
    u = basis^T @ G            PE, 3-term fp16 split (fp32-grade accuracy)
    alpha = exp(u)             ACT
    m = 1 - alpha              GPSIMD/DVE tensor_scalar (split by tile)
    S = cumprod(m)             DVE/GPSIMD tensor_tensor_scan, fp32 state, fp16 out
    S^T                        PE fp16 transposes -> PSUM, ACT/DVE copy -> SBUF
    r = S^T.T @ [g_hi|g_lo]    PE fp16, accumulated over 4 gaussian chunks
Tiles are processed in pairs to amortize instruction overheads. Voxel slabs
are sharded across the 8 cores; per-gaussian parameters are replicated. Host
does the tiny per-gaussian precompute (quat->rot, 3x3 inverse, fp16 hi/lo
splits) in float64 and the final gather/deinterleave.
"""
import numpy as np

import concourse.bacc as bacc
import concourse.tile as tile
import concourse.mybir as mybir
from concourse.bass_utils import run_bass_kernel_spmd
from concourse.masks import make_identity

F32 = mybir.dt.float32
F16 = mybir.dt.float16
AF = mybir.ActivationFunctionType
ALU = mybir.AluOpType

H, W, D = 96, 96, 16
N, F = 512, 32
NCORES = 8
P_TOTAL = H * W * D
P_LOCAL = P_TOTAL // NCORES          # 18432
TILES = P_LOCAL // 128               # 144
NCHUNK = N // 128                    # 4
LO_SCALE = 4096.0                    # 2**12, fp16 low-part scaling

# tunables (balanced via TimelineSim sweep)
GROUP = 6          # tiles per r_ps bank / out-copy batch (divides tpq=36)
OUT_CHUNKS = 4     # output DMA granularity
ACT_ST = 448      # columns (of 2N per tile-pair) of the S^T copy done by ACT
POOL_SCAN = 0      # GPSIMD scan rejected by compiler - keep on DVE
POOL_OM = 8        # of 8 consecutive tiles, how many run 1-alpha on GPSIMD


def _build_nc(act_st=None, pool_scan=None, pool_om=None, group=None, wbufs=3, ubufs=2):
    act_st = ACT_ST if act_st is None else act_st
    pool_scan = POOL_SCAN if pool_scan is None else pool_scan
    pool_om = POOL_OM if pool_om is None else pool_om
    group = GROUP if group is None else group

    nc = bacc.Bacc("TRN2", target_bir_lowering=False, debug=False)
    bcat_d = nc.dram_tensor("basis_cat", [30, P_LOCAL], F16,
                            kind="ExternalInput")
    gcat_d = nc.dram_tensor("G_cat", [30, N], F16, kind="ExternalInput")
    gf_d = nc.dram_tensor("gfeat", [128, NCHUNK * 2 * F], F16,
                          kind="ExternalInput")
    rend_d = nc.dram_tensor("rend", [128, TILES * F], F32, kind="ExternalOutput")

    tpq = TILES // OUT_CHUNKS
    with tile.TileContext(nc) as tc:
        with tc.tile_pool(name="const", bufs=1) as const, \
             tc.tile_pool(name="work", bufs=wbufs) as work, \
             tc.tile_pool(name="outp", bufs=2) as outp, \
             tc.tile_pool(name="ps_u", bufs=ubufs, space="PSUM") as ps_u, \
             tc.tile_pool(name="ps_t", bufs=2, space="PSUM") as ps_t, \
             tc.tile_pool(name="ps_r", bufs=2, space="PSUM") as ps_r:

            bcat_sb = const.tile([30, P_LOCAL], F16)
            nc.sync.dma_start(bcat_sb[:], bcat_d[:])
            gcat_sb = const.tile([30, N], F16)
            nc.sync.dma_start(gcat_sb[:], gcat_d[:])
            gf_sb = const.tile([128, NCHUNK * 2 * F], F16)
            nc.sync.dma_start(gf_sb[:], gf_d[:])
            ident = const.tile([128, 128], F16)
            make_identity(nc, ident[:])

            for q in range(OUT_CHUNKS):
                out_sb = outp.tile([128, tpq * F], F32, tag="out")
                for grp in range(tpq // group):
                    r_ps = ps_r.tile([128, group * 2 * F], F32, tag="r")
                    for pj in range(group // 2):
                        # process a pair of tiles together
                        jj = [grp * group + 2 * pj, grp * group + 2 * pj + 1]
                        ii = [q * tpq + j for j in jj]
                        u_ps = ps_u.tile([128, 2 * N], F32, tag="u")
                        for k in (0, 1):
                            sl = slice(ii[k] * 128, (ii[k] + 1) * 128)
                            nc.tensor.matmul(u_ps[:, k * N:(k + 1) * N],
                                             bcat_sb[:, sl], gcat_sb[:],
                                             start=True, stop=True)
                        alpha = work.tile([128, 2 * N], F32, tag="alpha")
                        nc.scalar.activation(alpha[:], u_ps[:], AF.Exp)
                        m = work.tile([128, 2 * N], F32, tag="m")
                        if (ii[0] // 2) % 4 < pool_om // 2:
                            nc.gpsimd.tensor_scalar(m[:], alpha[:], -1.0, 1.0,
                                                    op0=ALU.mult, op1=ALU.add)
                        else:
                            nc.vector.tensor_scalar(m[:], alpha[:], -1.0, 1.0,
                                                    op0=ALU.mult, op1=ALU.add)
                        S = work.tile([128, 2 * N], F16, tag="S")
                        for k in (0, 1):
                            on_pool = (ii[k] % 8) >= 8 - pool_scan
                            eng = nc.gpsimd if on_pool else nc.vector
                            eng.tensor_tensor_scan(
                                S[:, k * N:(k + 1) * N],
                                m[:, k * N:(k + 1) * N],
                                m[:, k * N:(k + 1) * N], 1.0,
                                op0=ALU.mult, op1=ALU.bypass)
                        st_ps = ps_t.tile([128, 2 * N], F16, tag="st")
                        for c in range(2 * NCHUNK):
                            nc.tensor.transpose(
                                st_ps[:, c * 128:(c + 1) * 128],
                                S[:, c * 128:(c + 1) * 128], ident[:])
                        ST = work.tile([128, 2 * N], F16, tag="ST")
                        if act_st > 0:
                            nc.scalar.activation(ST[:, 0:act_st],
                                                 st_ps[:, 0:act_st], AF.Copy)
                        if act_st < 2 * N:
                            nc.vector.tensor_copy(ST[:, act_st:2 * N],
                                                  st_ps[:, act_st:2 * N])
                        for k in (0, 1):
                            j = jj[k]
                            for c in range(NCHUNK):
                                nc.tensor.matmul(
                                    r_ps[:, (j % group) * 2 * F:
                                         (j % group + 1) * 2 * F],
                                    ST[:, (k * NCHUNK + c) * 128:
                                       (k * NCHUNK + c + 1) * 128],
                                    gf_sb[:, c * 2 * F:(c + 1) * 2 * F],
                                    start=(c == 0), stop=(c == NCHUNK - 1))
                    osl = out_sb[:, grp * group * F:(grp + 1) * group * F]
                    hi_view = r_ps[:].rearrange("p (grp two f) -> p grp two f",
                                                two=2, f=F)[:, :, 0, :]
                    lo_view = r_ps[:].rearrange("p (grp two f) -> p grp two f",
                                                two=2, f=F)[:, :, 1, :]
                    nc.scalar.activation(osl, hi_view, AF.Copy)
                    nc.vector.scalar_tensor_tensor(
                        osl, lo_view, 1.0 / LO_SCALE, osl,
                        op0=ALU.mult, op1=ALU.add)
                nc.sync.dma_start(rend_d[:, q * tpq * F:(q + 1) * tpq * F],
                                  out_sb[:])
    nc.compile()
    return nc


_NC_CACHE = None


def _get_nc():
    global _NC_CACHE
    if _NC_CACHE is None:
        _NC_CACHE = _build_nc()
    return _NC_CACHE


def _host_prep(means, scales, rotations, opacities, features, camera_transform,
               coord_grid):
    f8 = np.float64
    means = means.astype(f8)
    scales = scales.astype(f8)
    q = rotations.astype(f8)
    opa = opacities.astype(f8)[:, 0]
    T = camera_transform.astype(f8)

    homo = np.concatenate([means, np.ones((N, 1))], axis=1) @ T.T
    mu = homo[:, :3] / homo[:, 3:4]

    q = q / np.linalg.norm(q, axis=1, keepdims=True)
    w, x, y, z = q[:, 0], q[:, 1], q[:, 2], q[:, 3]
    R = np.stack([
        np.stack([1 - 2 * (y * y + z * z), 2 * (x * y - w * z), 2 * (x * z + w * y)], 1),
        np.stack([2 * (x * y + w * z), 1 - 2 * (x * x + z * z), 2 * (y * z - w * x)], 1),
        np.stack([2 * (x * z - w * y), 2 * (y * z + w * x), 1 - 2 * (x * x + y * y)], 1),
    ], axis=1)
    RS = R * scales[:, None, :]
    cov = np.einsum('nik,njk->nij', RS, RS)
    A = np.linalg.inv(cov)

    Am = np.einsum('nij,nj->ni', A, mu)
    const = -0.5 * np.einsum('ni,ni->n', mu, Am) + np.log(np.maximum(opa, 1e-300))
    G = np.empty((10, N), f8)
    G[0] = -0.5 * A[:, 0, 0]
    G[1] = -0.5 * A[:, 1, 1]
    G[2] = -0.5 * A[:, 2, 2]
    G[3] = -A[:, 0, 1]
    G[4] = -A[:, 0, 2]
    G[5] = -A[:, 1, 2]
    G[6] = Am[:, 0]
    G[7] = Am[:, 1]
    G[8] = Am[:, 2]
    G[9] = np.maximum(const, -60000.0)   # keep within fp16 range

    coords = coord_grid.astype(f8).reshape(-1, 3)
    cx, cy, cz = coords[:, 0], coords[:, 1], coords[:, 2]
    basis = np.stack([cx * cx, cy * cy, cz * cz, cx * cy, cx * cz, cy * cz,
                      cx, cy, cz, np.ones_like(cx)], axis=0)  # [10, P]

    h16 = np.float16
    b_hi = basis.astype(h16)
    b_lo = ((basis - b_hi.astype(f8)) * LO_SCALE).astype(h16)
    G_hi = G.astype(h16)
    G_lo = (G - G_hi.astype(f8)).astype(h16)
    G_his = (G_hi.astype(f8) / LO_SCALE).astype(h16)
    b_cat = np.concatenate([b_hi, b_hi, b_lo], axis=0)       # [30, P]
    G_cat = np.concatenate([G_hi, G_lo, G_his], axis=0)      # [30, N]

    feats = features.astype(f8)
    g = np.empty_like(feats)
    g[:-1] = feats[1:] - feats[:-1]
    g[-1] = -feats[-1]
    g_dev = g.reshape(NCHUNK, 128, F).transpose(1, 0, 2)      # [128, NCHUNK, F]
    gf_hi = g_dev.astype(h16)
    gf_lo = ((g_dev - gf_hi.astype(f8)) * LO_SCALE).astype(h16)
    gf = np.ascontiguousarray(
        np.concatenate([gf_hi[:, :, None, :], gf_lo[:, :, None, :]], axis=2)
        .reshape(128, NCHUNK * 2 * F))
    f0 = feats[0]

    return b_cat, G_cat, gf, f0.astype(np.float32)


def kernel(means, scales, rotations, opacities, features, camera_transform,
           coord_grid):
    b_cat, G_cat, gf, f0 = _host_prep(
        means, scales, rotations, opacities, features, camera_transform,
        coord_grid)
    nc = _get_nc()
    in_maps = []
    for c in range(NCORES):
        sl = slice(c * P_LOCAL, (c + 1) * P_LOCAL)
        in_maps.append({
            "basis_cat": np.ascontiguousarray(b_cat[:, sl]),
            "G_cat": G_cat, "gfeat": gf,
        })
    res = run_bass_kernel_spmd(nc, in_maps, core_ids=list(range(NCORES)))
    parts = []
    for c in range(NCORES):
        r = res.results[c]["rend"]                      # [128, TILES*F]
        part = r.reshape(128, TILES, F).transpose(1, 0, 2).reshape(P_LOCAL, F)
        parts.append(part)
    out = np.concatenate(parts, axis=0) + f0[None, :]
    return out.reshape(H, W, D, F).astype(np.float32)
